# revision 1
# baseline (speedup 1.0000x reference)
"""Trainium2 Bass kernel for nn_ChannelAttnBlock (GroupNorm + channel attention).

Self-contained: takes FULL unsharded inputs, shards batch over 8 NeuronCores
(2 batches/core), runs one SPMD NEFF, gathers the full output.

Per-core dataflow (B=2 batches, C=512 channels, T=8192):
  pass 0: stream x, bn_stats/bn_aggr -> per-channel mean/var; tiny selector
          matmuls aggregate the 32 GN groups -> per-channel affine a, b.
  pass A: h = a*x+b (ACT); qT/kT = h^T @ Wq/Wk (t-on-partitions, so the
          channel softmax is a free-dim segment reduce); exp on ACT; both
          softmax denominators folded into k' = e^k/(Sq*Sk); accumulate
          w = sum_t e^q k'^T in PSUM (per-head 32x32 diagonal blocks).
  pass B: recompute h; v = Wv h (head-permuted for packing); h2 = w @ v via
          4-way col-packed K=32 matmuls; out = Wp h2 + bp + x.
All big matmuls run in float32r (one-pass FP22 multiply, fp32 accumulate).
"""

import numpy as np

C = 512
NH = 16      # heads
HC = 32      # channels/head
G = 32       # groupnorm groups
CG = C // G  # 16 channels per group
EPS = 1e-6

N_CORES = 8
B_FULL = 16
T_FULL = 8192
B_SHARD = B_FULL // N_CORES  # 2
TT = 512                     # t macro-tile
NM = T_FULL // TT            # 16 macros per batch


def _head_perm_v():
    # v-ctile m holds heads (m, m+4, m+8, m+12) at 32-row slots 0..3
    pv = np.zeros(C, dtype=np.int64)
    for h in range(NH):
        m, s = h % 4, h // 4
        pv[128 * m + 32 * s: 128 * m + 32 * s + 32] = np.arange(32 * h, 32 * h + 32)
    return pv


def _to_part4(vec):
    # [512] -> [128, 4]: column j = channels 128j..128j+127
    return np.ascontiguousarray(vec.reshape(4, 128).T)

def build_nc(B, T, qk_bias=True, debug=False, dbg_taps=False):
    import concourse.tile as tile
    import concourse.mybir as mybir
    from concourse import bacc

    NMi = T // TT
    f32 = mybir.dt.float32
    f32r = mybir.dt.float32r
    AF = mybir.ActivationFunctionType
    ALU = mybir.AluOpType
    AX = mybir.AxisListType

    nc = bacc.Bacc("TRN2", target_bir_lowering=False, debug=debug)

    dbg = {}
    if dbg_taps:
        for name, shape in [
            ("dbg_a", [128, 4]), ("dbg_b", [128, 4]),
            ("dbg_h", [128, 4, TT]), ("dbg_eq", [128, TT]),
            ("dbg_kp", [128, TT]), ("dbg_wsb", [128, 4, 128]),
            ("dbg_v", [128, 4, TT]), ("dbg_h2", [128, 4, TT]),
            ("dbg_wps", [128, 4, 128]), ("dbg_wsc", [128, 4, 128]),
        ]:
            dbg[name] = nc.dram_tensor(name, shape, f32,
                                       kind="ExternalOutput").ap()

    def tap(name, ap):
        if dbg_taps and name in dbg:
            nc.sync.dma_start(dbg[name], ap)

    x_d = nc.dram_tensor("x", [B, C, T], f32, kind="ExternalInput").ap()
    wqt_d = nc.dram_tensor("wqt", [C, C], f32, kind="ExternalInput").ap()
    wkt_d = nc.dram_tensor("wkt", [C, C], f32, kind="ExternalInput").ap()
    wvt_d = nc.dram_tensor("wvt", [C, C], f32, kind="ExternalInput").ap()
    wpt_d = nc.dram_tensor("wpt", [C, C], f32, kind="ExternalInput").ap()
    gammaP_d = nc.dram_tensor("gammaP", [128, 4], f32, kind="ExternalInput").ap()
    betaP_d = nc.dram_tensor("betaP", [128, 4], f32, kind="ExternalInput").ap()
    bq_row_d = nc.dram_tensor("bq_row", [1, C], f32, kind="ExternalInput").ap()
    bk_row_d = nc.dram_tensor("bk_row", [1, C], f32, kind="ExternalInput").ap()
    bvP_d = nc.dram_tensor("bvP", [128, 4], f32, kind="ExternalInput").ap()
    bpP_d = nc.dram_tensor("bpP", [128, 4], f32, kind="ExternalInput").ap()
    sel_d = nc.dram_tensor("sel", [128, 8], f32, kind="ExternalInput").ap()
    selT_d = nc.dram_tensor("selT", [8, 128], f32, kind="ExternalInput").ap()
    ones1_d = nc.dram_tensor("ones1", [1, 128], f32, kind="ExternalInput").ap()
    out_d = nc.dram_tensor("out", [B, C, T], f32, kind="ExternalOutput").ap()

    def r(ap):
        return ap.bitcast(f32r)

    from contextlib import ExitStack

    with tile.TileContext(nc) as tc, ExitStack() as est:
        p = lambda name, bufs: est.enter_context(
            tc.tile_pool(name=name, bufs=bufs))
        wpool = p("wpool", 1)
        cpool = p("cpool", 1)
        stpool = p("stpool", 2)
        xin = p("xin", 3)
        hpool = p("hpool", 2)
        eqpool = p("eqpool", 2)
        ekpool = p("ekpool", 2)
        kppool = p("kppool", 2)
        smpool = p("smpool", 4)
        wsbpool = p("wsb", 2)
        wfpool = p("wfpool", 2)
        vpool = p("vpool", 2)
        h2pool = p("h2pool", 2)
        opool = p("opool", 2)

        # ---- load weights & constants ----
        wqt_sb = wpool.tile([128, 4, C], f32)
        wkt_sb = wpool.tile([128, 4, C], f32)
        wvt_sb = wpool.tile([128, 4, C], f32)
        wpt_sb = wpool.tile([128, 4, C], f32)
        for j in range(4):
            nc.sync.dma_start(r(wqt_sb[:, j, :]),
                              r(wqt_d[128 * j:128 * j + 128, :]))
            nc.sync.dma_start(r(wkt_sb[:, j, :]),
                              r(wkt_d[128 * j:128 * j + 128, :]))
            nc.sync.dma_start(wvt_sb[:, j, :],
                              wvt_d[128 * j:128 * j + 128, :])
            nc.sync.dma_start(r(wpt_sb[:, j, :]),
                              r(wpt_d[128 * j:128 * j + 128, :]))
        gammaP = cpool.tile([128, 4], f32)
        betaP = cpool.tile([128, 4], f32)
        bvP = cpool.tile([128, 4], f32)
        bpP = cpool.tile([128, 4], f32)
        bq_row = cpool.tile([1, C], f32)
        bk_row = cpool.tile([1, C], f32)
        sel_sb = cpool.tile([128, 8], f32)
        selT_sb = cpool.tile([8, 128], f32)
        ones1 = cpool.tile([1, 128], f32)
        nc.sync.dma_start(gammaP[:], gammaP_d)
        nc.sync.dma_start(betaP[:], betaP_d)
        nc.sync.dma_start(bvP[:], bvP_d)
        nc.sync.dma_start(bpP[:], bpP_d)
        nc.sync.dma_start(r(bq_row[:]), r(bq_row_d))
        nc.sync.dma_start(r(bk_row[:]), r(bk_row_d))
        nc.sync.dma_start(sel_sb[:], sel_d)
        nc.sync.dma_start(selT_sb[:], selT_d)
        nc.sync.dma_start(r(ones1[:]), r(ones1_d))
        eps_t = cpool.tile([8, 1], f32)
        nc.vector.memset(eps_t[:], EPS)
        zeros_t = cpool.tile([128, 4, 128], f32)
        nc.vector.memset(zeros_t[:], 0.0)

        bn_tiles = {}
        ab_tiles = {}

        def x_macro_ap(b, i):
            return x_d[b, :, TT * i:TT * i + TT].rearrange(
                "(j p) t -> p j t", p=128)

        def emit_pass0_macro(b, i):
            if b not in bn_tiles:
                bnall = stpool.tile([128, 4, NMi * 6], f32, tag="bnall",
                                    name=f"bnall{b}")
                bn_tiles[b] = bnall
            bnall = bn_tiles[b]
            xt = xin.tile([128, 4, TT], f32, tag="xt", name=f"x0_{b}_{i}")
            nc.sync.dma_start(xt[:], x_macro_ap(b, i))
            for j in range(4):
                nc.vector.bn_stats(bnall[:, j, 6 * i:6 * i + 6], xt[:, j, :])

        def emit_finalize(b):
            bnall = bn_tiles[b]
            statsc = stpool.tile([128, 4, 2], f32, tag="statsc",
                                 name=f"statsc{b}")
            stats2 = stpool.tile([128, 8], f32, tag="stats2",
                                 name=f"stats2_{b}")
            for j in range(4):
                nc.vector.bn_aggr(statsc[:, j, :], bnall[:, j, :])
                nc.vector.tensor_copy(stats2[:, 2 * j:2 * j + 1],
                                      statsc[:, j, 0:1])
                nc.vector.scalar_tensor_tensor(
                    stats2[:, 2 * j + 1:2 * j + 2],
                    in0=statsc[:, j, 0:1], scalar=statsc[:, j, 0:1],
                    in1=statsc[:, j, 1:2], op0=ALU.mult, op1=ALU.add)
            aT = stpool.tile([128, 4], f32, tag="aT", name=f"aT{b}")
            bvec = stpool.tile([128, 4], f32, tag="bvec", name=f"bvec{b}")
            with tc.tile_pool(name=f"st_ps{b}", bufs=2, space="PSUM") as stps:
                gsum_ps = stps.tile([8, 8], f32, name=f"gsum{b}")
                nc.tensor.matmul(gsum_ps[:], sel_sb[:], stats2[:])
                gs = stpool.tile([8, 4, 2], f32, tag="gs", name=f"gs{b}")
                nc.vector.tensor_scalar_mul(gs[:], gsum_ps.rearrange(
                    "p (j s) -> p j s", s=2), 1.0 / CG)
                mg2 = stpool.tile([8, 4], f32, tag="mg2", name=f"mg2_{b}")
                nc.vector.tensor_mul(mg2[:], gs[:, :, 0], gs[:, :, 0])
                gvar = stpool.tile([8, 4], f32, tag="gvar", name=f"gvar{b}")
                nc.vector.tensor_sub(gvar[:], gs[:, :, 1], mg2[:])
                gstd = stpool.tile([8, 4], f32, tag="gstd", name=f"gstd{b}")
                nc.scalar.activation(gstd[:], gvar[:], AF.Sqrt, bias=eps_t[:])
                ginv = stpool.tile([8, 4], f32, tag="ginv", name=f"ginv{b}")
                nc.vector.reciprocal(ginv[:], gstd[:])
                gb = stpool.tile([8, 4, 2], f32, tag="gb", name=f"gb{b}")
                nc.vector.tensor_copy(gb[:, :, 0], gs[:, :, 0])
                nc.vector.tensor_copy(gb[:, :, 1], ginv[:])
                chB_ps = stps.tile([128, 8], f32, name=f"chB{b}")
                nc.tensor.matmul(chB_ps[:], selT_sb[:], gb.rearrange(
                    "p j s -> p (j s)"))
                chB = chB_ps.rearrange("p (j s) -> p j s", s=2)
                nc.vector.tensor_mul(aT[:], gammaP[:], chB[:, :, 1])
                tmpb = stpool.tile([128, 4], f32, tag="tmpb", name=f"tmpb{b}")
                nc.vector.tensor_mul(tmpb[:], chB[:, :, 0], aT[:])
                nc.vector.tensor_sub(bvec[:], betaP[:], tmpb[:])
            # pass-B folded v-weights: wv_f = wvt * a (per input-channel row),
            # cv = wv^T b + bv (per output channel)
            wv_f = wfpool.tile([128, 4, C], f32, tag="wvf", name=f"wvf{b}")
            for j in range(4):
                nc.vector.tensor_scalar_mul(r(wv_f[:, j, :]), wvt_sb[:, j, :],
                                            aT[:, j:j + 1])
            cvP = stpool.tile([128, 4], f32, tag="cvP", name=f"cvP{b}")
            with tc.tile_pool(name=f"cv_ps{b}", bufs=1, space="PSUM") as cvps:
                cv_ps = cvps.tile([128, 4], f32, name=f"cvp{b}")
                for m in range(4):
                    for j in range(4):
                        nc.tensor.matmul(
                            cv_ps[:, m:m + 1],
                            wvt_sb[:, j, 128 * m:128 * m + 128],
                            bvec[:, j:j + 1],
                            start=(j == 0 and m == 0),
                            stop=(j == 3 and m == 3),
                            skip_group_check=True)
                nc.vector.tensor_add(cvP[:], cv_ps[:], bvP[:])
            if b == 0:
                tap("dbg_a", aT[:])
                tap("dbg_b", bvec[:])
            ab_tiles[b] = (aT, bvec, wv_f, cvP)

        def emit_passA(b, interleave_next):
            aT, bvec, _, _ = ab_tiles[b]
            with ExitStack() as est_a:
                qps_pool = est_a.enter_context(
                    tc.tile_pool(name=f"q_ps{b}", bufs=3, space="PSUM"))
                kps_pool = est_a.enter_context(
                    tc.tile_pool(name=f"k_ps{b}", bufs=3, space="PSUM"))
                wps_pool = est_a.enter_context(
                    tc.tile_pool(name=f"w_ps{b}", bufs=1, space="PSUM"))
                w_ps = wps_pool.tile([128, 4, 128], f32, name=f"wps{b}")
                for i in range(NMi):
                    xt = xin.tile([128, 4, TT], f32, tag="xt",
                                  name=f"xa_{b}_{i}")
                    nc.sync.dma_start(xt[:], x_macro_ap(b, i))
                    ht = hpool.tile([128, 4, TT], f32, tag="ht",
                                    name=f"ha_{b}_{i}")
                    for j in range(4):
                        nc.scalar.activation(
                            r(ht[:, j, :]), xt[:, j, :], AF.Identity,
                            bias=bvec[:, j:j + 1], scale=aT[:, j:j + 1])
                    if b == 0 and i == 0:
                        tap("dbg_h", ht[:])
                    eq = eqpool.tile([128, 4, TT], f32, tag="eq",
                                     name=f"eq_{b}_{i}")
                    ek = ekpool.tile([128, 4, TT], f32, tag="ek",
                                     name=f"ek_{b}_{i}")
                    for s in range(4):
                        qps = qps_pool.tile([128, TT], f32, tag="q",
                                            name=f"qps_{b}_{i}_{s}")
                        kps = kps_pool.tile([128, TT], f32, tag="k",
                                            name=f"kps_{b}_{i}_{s}")
                        for j in range(4):
                            lhs = r(ht[:, j, 128 * s:128 * s + 128])
                            nc.tensor.matmul(
                                qps[:], lhs, r(wqt_sb[:, j, :]),
                                start=(j == 0),
                                stop=(j == 3 and not qk_bias))
                            nc.tensor.matmul(
                                kps[:], lhs, r(wkt_sb[:, j, :]),
                                start=(j == 0),
                                stop=(j == 3 and not qk_bias))
                        if qk_bias:
                            nc.tensor.matmul(
                                qps[:], r(ones1[:]), r(bq_row[:]),
                                start=False, stop=True)
                            nc.tensor.matmul(
                                kps[:], r(ones1[:]), r(bk_row[:]),
                                start=False, stop=True)
                        nc.scalar.activation(r(eq[:, s, :]), qps[:], AF.Exp)
                        nc.scalar.activation(r(ek[:, s, :]), kps[:], AF.Exp)
                    sq = smpool.tile([128, 4 * NH], f32, tag="sq",
                                     name=f"sq_{b}_{i}")
                    nc.vector.tensor_reduce(
                        sq[:], eq.rearrange("p s (n c) -> p s n c", c=HC),
                        axis=AX.X, op=ALU.add)
                    sk = smpool.tile([128, 4 * NH], f32, tag="sk",
                                     name=f"sk_{b}_{i}")
                    nc.vector.tensor_reduce(
                        sk[:], ek.rearrange("p s (n c) -> p s n c", c=HC),
                        axis=AX.X, op=ALU.add)
                    ss = smpool.tile([128, 4 * NH], f32, tag="ss",
                                     name=f"ss_{b}_{i}")
                    nc.vector.tensor_mul(ss[:], sq[:], sk[:])
                    rr = smpool.tile([128, 4 * NH], f32, tag="rr",
                                     name=f"rr_{b}_{i}")
                    nc.vector.reciprocal(rr[:], ss[:])
                    kp = kppool.tile([128, 4, TT], f32, tag="kp",
                                     name=f"kp_{b}_{i}")
                    nc.vector.tensor_mul(
                        r(kp).rearrange("p s (n c) -> p s n c", c=HC),
                        ek.rearrange("p s (n c) -> p s n c", c=HC),
                        rr.rearrange("p (s n) -> p s n", s=4)[
                            :, :, :, None].broadcast_to([128, 4, NH, HC]))
                    if b == 0 and i == 0:
                        tap("dbg_eq", eq[:, 0, :])
                        tap("dbg_kp", kp[:, 0, :])
                    for s in range(4):
                        first = (i == 0 and s == 0)
                        last = (i == NMi - 1 and s == 3)
                        for m in range(4):
                            # only the first MM into the bank may set start
                            # (start zeroes the whole 2KB PSUM zero region)
                            nc.tensor.matmul(
                                w_ps[:, m, :],
                                r(kp[:, s, 128 * m:128 * m + 128]),
                                r(eq[:, s, 128 * m:128 * m + 128]),
                                start=(first and m == 0),
                                stop=(last and m == 3),
                                skip_group_check=True)
                    if interleave_next is not None:
                        emit_pass0_macro(interleave_next, i)
                # w finalize: PSUM -> SBUF, then per-head 32x32 blocks into
                # block-diagonal w_sb (head h=4s+m at [32s, m, 32s])
                w_sc = wsbpool.tile([128, 4, 128], f32, tag="wsc",
                                    name=f"wsc{b}")
                nc.vector.tensor_copy(w_sc[:], w_ps[:])
                if b == 0:
                    tap("dbg_wps", w_sc[:])
                    tap("dbg_wsc", w_sc[:])
            w_sb = wsbpool.tile([128, 4, 128], f32, tag="wsb",
                                name=f"wsb{b}")
            nc.sync.dma_start(r(w_sb[:]), r(zeros_t[:]))
            for h in range(NH):
                s, m = h // 4, h % 4
                nc.sync.dma_start(
                    r(w_sb[32 * s:32 * s + 32, m, 32 * s:32 * s + 32]),
                    r(w_sc[32 * m:32 * m + 32, s, 32 * m:32 * m + 32]))
            if b == 0:
                tap("dbg_wsb", w_sb[:])
            return w_sb

        def emit_passB(b, w_sb):
            aT, bvec, wv_f, cvP = ab_tiles[b]
            with ExitStack() as est_b:
                vps_pool = est_b.enter_context(
                    tc.tile_pool(name=f"v_ps{b}", bufs=2, space="PSUM"))
                h2ps_pool = est_b.enter_context(
                    tc.tile_pool(name=f"h2_ps{b}", bufs=4, space="PSUM"))
                pjps_pool = est_b.enter_context(
                    tc.tile_pool(name=f"pj_ps{b}", bufs=2, space="PSUM"))
                for i in range(NMi):
                    xt = xin.tile([128, 4, TT], f32, tag="xt",
                                  name=f"xb_{b}_{i}")
                    nc.sync.dma_start(r(xt[:]), r(x_macro_ap(b, i)))
                    vsb = vpool.tile([128, 4, TT], f32, tag="vsb",
                                     name=f"vsb_{b}_{i}")
                    for m in range(4):
                        vps = vps_pool.tile([128, TT], f32, tag="v",
                                            name=f"vps_{b}_{i}_{m}")
                        for j in range(4):
                            nc.tensor.matmul(
                                vps[:],
                                r(wv_f[:, j, 128 * m:128 * m + 128]),
                                r(xt[:, j, :]),
                                start=(j == 0), stop=(j == 3))
                        nc.scalar.activation(
                            r(vsb[:, m, :]), vps[:], AF.Identity,
                            bias=cvP[:, m:m + 1])
                    if b == 0 and i == 0:
                        tap("dbg_v", vsb[:])
                    h2t = h2pool.tile([128, 4, TT], f32, tag="h2",
                                      name=f"h2_{b}_{i}")
                    for m in range(4):
                        h2ps = h2ps_pool.tile([128, TT], f32, tag="h2p",
                                              name=f"h2ps_{b}_{i}_{m}")
                        nc.tensor.matmul(
                            h2ps[:], r(w_sb[:, m, :]), r(vsb[:, m, :]))
                        if m % 2 == 0:
                            nc.vector.tensor_copy(r(h2t[:, m, :]), h2ps[:])
                        else:
                            nc.scalar.copy(r(h2t[:, m, :]), h2ps[:])
                    if b == 0 and i == 0:
                        tap("dbg_h2", h2t[:])
                    ot = opool.tile([128, 4, TT], f32, tag="ot",
                                    name=f"ot_{b}_{i}")
                    for n in range(4):
                        pj = pjps_pool.tile([128, TT], f32, tag="pj",
                                            name=f"pj_{b}_{i}_{n}")
                        for g in range(4):
                            nc.tensor.matmul(
                                pj[:],
                                r(wpt_sb[:, g, 128 * n:128 * n + 128]),
                                r(h2t[:, g, :]),
                                start=(g == 0), stop=(g == 3))
                        nc.vector.scalar_tensor_tensor(
                            ot[:, n, :], in0=pj[:], scalar=bpP[:, n:n + 1],
                            in1=xt[:, n, :], op0=ALU.add, op1=ALU.add)
                    nc.sync.dma_start(
                        out_d[b, :, TT * i:TT * i + TT].rearrange(
                            "(j p) t -> p j t", p=128),
                        ot[:])

        # schedule: pass0(0); then per batch: finalize, passA (with next
        # batch's stats pass interleaved), w-rearrange, passB.
        for i in range(NMi):
            emit_pass0_macro(0, i)
        for b in range(B):
            emit_finalize(b)
            w_sb = emit_passA(b, b + 1 if b + 1 < B else None)
            emit_passB(b, w_sb)

    nc.compile()
    return nc
def _host_prep(x, gn_scale, gn_bias, wq, bq, wk, bk, wv, bv, wp, bp):
    pv = _head_perm_v()
    sel = np.zeros((128, 8), dtype=np.float32)
    for p in range(128):
        sel[p, p // CG] = 1.0
    consts = {
        "wqt": np.ascontiguousarray(wq.T).astype(np.float32),
        "wkt": np.ascontiguousarray(wk.T).astype(np.float32),
        "wvt": np.ascontiguousarray(wv.T[:, pv]).astype(np.float32),
        "wpt": np.ascontiguousarray(wp.T[pv, :]).astype(np.float32),
        "gammaP": _to_part4(gn_scale).astype(np.float32),
        "betaP": _to_part4(gn_bias).astype(np.float32),
        "bq_row": bq.reshape(1, C).astype(np.float32),
        "bk_row": bk.reshape(1, C).astype(np.float32),
        "bvP": _to_part4(bv[pv]).astype(np.float32),
        "bpP": _to_part4(bp).astype(np.float32),
        "sel": sel,
        "selT": np.ascontiguousarray(sel.T),
        "ones1": np.ones((1, 128), dtype=np.float32),
    }
    return consts


_NC_CACHE = {}


def kernel(x, gn_scale, gn_bias, wq, bq, wk, bk, wv, bv, wp, bp):
    from concourse.bass_utils import run_bass_kernel_spmd

    x = np.asarray(x, dtype=np.float32)
    consts = _host_prep(x, np.asarray(gn_scale), np.asarray(gn_bias),
                        np.asarray(wq), np.asarray(bq), np.asarray(wk),
                        np.asarray(bk), np.asarray(wv), np.asarray(bv),
                        np.asarray(wp), np.asarray(bp))

    qk_bias = bool(np.any(np.asarray(bq)) or np.any(np.asarray(bk)))
    key = (B_SHARD, T_FULL, qk_bias)
    if key not in _NC_CACHE:
        _NC_CACHE[key] = build_nc(B_SHARD, T_FULL, qk_bias=qk_bias)
    nc = _NC_CACHE[key]

    in_maps = []
    for c in range(N_CORES):
        m = dict(consts)
        m["x"] = np.ascontiguousarray(x[B_SHARD * c:B_SHARD * (c + 1)])
        in_maps.append(m)
    res = run_bass_kernel_spmd(nc, in_maps, core_ids=list(range(N_CORES)))
    out = np.concatenate([r["out"] for r in res.results], axis=0)
    return out.astype(np.float32)



# revision 12
# speedup vs baseline: 1.0885x; 1.0885x over previous
"""Trainium2 Bass kernel for nn_ChannelAttnBlock (GroupNorm + channel attention).

Self-contained: takes FULL unsharded inputs, shards batch over 8 NeuronCores
(2 batches/core), runs one SPMD NEFF, gathers the full output.

Per-core dataflow (B=2 batches, C=512, T=8192), v2:
  pass 0: stream x (fp32) once from HBM; bn_stats for GroupNorm; cast x to a
          bf16 SBUF-resident copy (xbf). No further HBM reads of x.
  finalize: tiny selector matmuls aggregate 32 GN groups -> per-channel
          affine a, b; fold a into bf16 v-weights (wv_f) and compute the
          folded v bias cv = Wv^T b + bv.
  pass A: h8 = fp8(a*xbf+b) on gpsimd; q/k GEMMs as fp8 DoubleRow matmuls
          (K=256 per instr); exp on ACT with output in bf16 (weight
          prescale WS undone via the ACT scale); channel softmax sums are
          free-dim segment reduces; kp = ek/(Sq*Sk) in place; w accumulated
          in PSUM with bf16 N=128 matmuls.
  pass B: v = wv_f^T xbf (bf16 matmuls); h2 = w @ v via block-diag packed
          bf16 w; out = Wp h2 + bp' + xbf with all biases pre-folded into
          bp' = bp + Wp(w@cv).
  schedule: pass0(0); passA(0) || pass0(1); passB(0) || passA(1); passB(1).
  PE emission is software-pipelined (w-mm of i-1 after qk of i, etc.) so the
  tensor engine never waits on ACT/DVE.
"""

import numpy as np
import ml_dtypes

C = 512
NH = 16      # heads
HC = 32      # channels/head
G = 32       # groupnorm groups
CG = C // G  # 16 channels per group
EPS = 1e-6
WS = 32.0    # q/k weight prescale before fp8 cast (undone in exp ACT)

N_CORES = 8
B_FULL = 16
T_FULL = 8192
B_SHARD = B_FULL // N_CORES  # 2
TT = 512                     # t macro-tile
NM = T_FULL // TT            # 16 macros per batch

F8NP = ml_dtypes.float8_e4m3
BF16NP = ml_dtypes.bfloat16


def _head_perm_v():
    # v-ctile m holds heads (m, m+4, m+8, m+12) at 32-row slots 0..3
    pv = np.zeros(C, dtype=np.int64)
    for h in range(NH):
        m, s = h % 4, h // 4
        pv[128 * m + 32 * s: 128 * m + 32 * s + 32] = np.arange(32 * h, 32 * h + 32)
    return pv


def _to_part4(vec):
    # [512] -> [128, 4]: column j = channels 128j..128j+127
    return np.ascontiguousarray(vec.reshape(4, 128).T)


def build_nc(B, T, debug=False):
    import concourse.tile as tile
    import concourse.mybir as mybir
    from concourse import bacc

    NMi = T // TT
    f32 = mybir.dt.float32
    f32r = mybir.dt.float32r
    f8 = mybir.dt.float8e4
    bf = mybir.dt.bfloat16
    AF = mybir.ActivationFunctionType
    ALU = mybir.AluOpType
    AX = mybir.AxisListType
    DR = mybir.MatmulPerfMode.DoubleRow

    nc = bacc.Bacc("TRN2", target_bir_lowering=False, debug=debug)

    x_d = nc.dram_tensor("x", [B, C, T], f32, kind="ExternalInput").ap()
    wqt8_d = nc.dram_tensor("wqt8", [C, C], f8, kind="ExternalInput").ap()
    wkt8_d = nc.dram_tensor("wkt8", [C, C], f8, kind="ExternalInput").ap()
    wvt16_d = nc.dram_tensor("wvt16", [C, C], bf, kind="ExternalInput").ap()
    wpt16_d = nc.dram_tensor("wpt16", [C, C], bf, kind="ExternalInput").ap()
    gammaP_d = nc.dram_tensor("gammaP", [128, 4], f32, kind="ExternalInput").ap()
    betaP_d = nc.dram_tensor("betaP", [128, 4], f32, kind="ExternalInput").ap()
    bvP_d = nc.dram_tensor("bvP", [128, 4], f32, kind="ExternalInput").ap()
    bpP_d = nc.dram_tensor("bpP", [128, 4], f32, kind="ExternalInput").ap()
    sel_d = nc.dram_tensor("sel", [128, 8], f32, kind="ExternalInput").ap()
    selT_d = nc.dram_tensor("selT", [8, 128], f32, kind="ExternalInput").ap()
    out_d = nc.dram_tensor("out", [B, C, T], f32, kind="ExternalOutput").ap()

    def r(ap):
        return ap.bitcast(f32r)

    from contextlib import ExitStack

    with tile.TileContext(nc) as tc, ExitStack() as est:
        p = lambda name, bufs: est.enter_context(
            tc.tile_pool(name=name, bufs=bufs))
        wpool = p("wpool", 1)
        cpool = p("cpool", 1)
        xbfpool = p("xbfpool", 1)
        stpool = p("stpool", 2)
        xin = p("xin", 2)       # fp32 staging: pass0 in, passB out
        h8pool = p("h8pool", 2)
        wka = p("wka", 5)       # passA bf16 work tiles (eq, ek)
        wkb = p("wkb", 3)       # passB bf16 work tiles (vsb, h2b)
        smpool = p("smpool", 2)
        wsbpool = p("wsbpool", 2)
        wscpool = p("wscpool", 1)
        wfpool = p("wfpool", 1)

        # ---- load weights & constants ----
        wqt8 = wpool.tile([128, 4, C], f8)
        wkt8 = wpool.tile([128, 4, C], f8)
        wvt16 = wpool.tile([128, 4, C], bf)
        wpt16 = wpool.tile([128, 4, C], bf)
        for j in range(4):
            nc.sync.dma_start(wqt8[:, j, :], wqt8_d[128 * j:128 * j + 128, :])
            nc.sync.dma_start(wkt8[:, j, :], wkt8_d[128 * j:128 * j + 128, :])
            nc.sync.dma_start(wvt16[:, j, :], wvt16_d[128 * j:128 * j + 128, :])
            nc.sync.dma_start(wpt16[:, j, :], wpt16_d[128 * j:128 * j + 128, :])
        gammaP = cpool.tile([128, 4], f32)
        betaP = cpool.tile([128, 4], f32)
        bvP = cpool.tile([128, 4], f32)
        bpP = cpool.tile([128, 4], f32)
        sel_sb = cpool.tile([128, 8], f32)
        selT_sb = cpool.tile([8, 128], f32)
        nc.sync.dma_start(gammaP[:], gammaP_d)
        nc.sync.dma_start(betaP[:], betaP_d)
        nc.sync.dma_start(bvP[:], bvP_d)
        nc.sync.dma_start(bpP[:], bpP_d)
        nc.sync.dma_start(sel_sb[:], sel_d)
        nc.sync.dma_start(selT_sb[:], selT_d)
        eps_t = cpool.tile([8, 1], f32)
        nc.vector.memset(eps_t[:], EPS)

        xbf = [xbfpool.tile([128, 4, T], bf, name=f"xbf{b}") for b in range(B)]

        bn_tiles = {}
        ab_tiles = {}
        fold_tiles = {}

        def x_macro_ap(b, i):
            return x_d[b, :, TT * i:TT * i + TT].rearrange(
                "(j p) t -> p j t", p=128)

        def emit_pass0_macro(b, i):
            if b not in bn_tiles:
                bnall = stpool.tile([128, 4, NMi * 6], f32, tag="bnall",
                                    name=f"bnall{b}")
                bn_tiles[b] = bnall
            bnall = bn_tiles[b]
            xt = xin.tile([128, 4, TT], f32, tag="xt", name=f"x0_{b}_{i}")
            nc.sync.dma_start(xt[:], x_macro_ap(b, i))
            # bf16 resident copy + GN stats (vector, fp32 input)
            nc.vector.tensor_copy(xbf[b][:, :, TT * i:TT * i + TT], xt[:])
            for j in range(4):
                nc.vector.bn_stats(bnall[:, j, 6 * i:6 * i + 6], xt[:, j, :])

        def emit_finalize(b):
            bnall = bn_tiles[b]
            statsc = stpool.tile([128, 4, 2], f32, tag="statsc",
                                 name=f"statsc{b}")
            stats2 = stpool.tile([128, 8], f32, tag="stats2",
                                 name=f"stats2_{b}")
            for j in range(4):
                nc.vector.bn_aggr(statsc[:, j, :], bnall[:, j, :])
                nc.vector.tensor_copy(stats2[:, 2 * j:2 * j + 1],
                                      statsc[:, j, 0:1])
                nc.vector.scalar_tensor_tensor(
                    stats2[:, 2 * j + 1:2 * j + 2],
                    in0=statsc[:, j, 0:1], scalar=statsc[:, j, 0:1],
                    in1=statsc[:, j, 1:2], op0=ALU.mult, op1=ALU.add)
            aT = stpool.tile([128, 4], f32, tag="aT", name=f"aT{b}")
            bvec = stpool.tile([128, 4], f32, tag="bvec", name=f"bvec{b}")
            with tc.tile_pool(name=f"st_ps{b}", bufs=2, space="PSUM") as stps:
                gsum_ps = stps.tile([8, 8], f32, name=f"gsum{b}")
                nc.tensor.matmul(gsum_ps[:], sel_sb[:], stats2[:])
                gs = stpool.tile([8, 4, 2], f32, tag="gs", name=f"gs{b}")
                nc.vector.tensor_scalar_mul(gs[:], gsum_ps.rearrange(
                    "p (j s) -> p j s", s=2), 1.0 / CG)
                mg2 = stpool.tile([8, 4], f32, tag="mg2", name=f"mg2_{b}")
                nc.vector.tensor_mul(mg2[:], gs[:, :, 0], gs[:, :, 0])
                gvar = stpool.tile([8, 4], f32, tag="gvar", name=f"gvar{b}")
                nc.vector.tensor_sub(gvar[:], gs[:, :, 1], mg2[:])
                gstd = stpool.tile([8, 4], f32, tag="gstd", name=f"gstd{b}")
                nc.scalar.activation(gstd[:], gvar[:], AF.Sqrt, bias=eps_t[:])
                ginv = stpool.tile([8, 4], f32, tag="ginv", name=f"ginv{b}")
                nc.vector.reciprocal(ginv[:], gstd[:])
                gb = stpool.tile([8, 4, 2], f32, tag="gb", name=f"gb{b}")
                nc.vector.tensor_copy(gb[:, :, 0], gs[:, :, 0])
                nc.vector.tensor_copy(gb[:, :, 1], ginv[:])
                chB_ps = stps.tile([128, 8], f32, name=f"chB{b}")
                nc.tensor.matmul(chB_ps[:], selT_sb[:], gb.rearrange(
                    "p j s -> p (j s)"))
                chB = chB_ps.rearrange("p (j s) -> p j s", s=2)
                nc.vector.tensor_mul(aT[:], gammaP[:], chB[:, :, 1])
                tmpb = stpool.tile([128, 4], f32, tag="tmpb", name=f"tmpb{b}")
                nc.vector.tensor_mul(tmpb[:], chB[:, :, 0], aT[:])
                nc.vector.tensor_sub(bvec[:], betaP[:], tmpb[:])
            # fold GN affine into v-weights: wv_f = wvt16 * a (per in-ch row)
            wv_f = wfpool.tile([128, 4, C], bf, tag="wvf", name=f"wvf{b}")
            for j in range(4):
                nc.vector.tensor_scalar_mul(wv_f[:, j, :], wvt16[:, j, :],
                                            aT[:, j:j + 1])
            # cv = Wv^T b + bv (per v-channel), bf16 copy for tiny matmuls
            bvec16 = stpool.tile([128, 4], bf, tag="bvec16", name=f"bv16_{b}")
            nc.vector.tensor_copy(bvec16[:], bvec[:])
            cvP16 = stpool.tile([128, 4], bf, tag="cvP16", name=f"cvP16_{b}")
            with tc.tile_pool(name=f"cv_ps{b}", bufs=1, space="PSUM") as cvps:
                cv_ps = cvps.tile([128, 4], f32, name=f"cvp{b}")
                for m in range(4):
                    for j in range(4):
                        nc.tensor.matmul(
                            cv_ps[:, m:m + 1],
                            wvt16[:, j, 128 * m:128 * m + 128],
                            bvec16[:, j:j + 1],
                            start=(j == 0 and m == 0),
                            stop=(j == 3 and m == 3),
                            skip_group_check=True)
                cv_f = stpool.tile([128, 4], f32, tag="cvf", name=f"cvf{b}")
                nc.vector.tensor_add(cv_f[:], cv_ps[:], bvP[:])
                nc.vector.tensor_copy(cvP16[:], cv_f[:])
            ab_tiles[b] = (aT, bvec, wv_f, cvP16)

        def emit_h8(b, i):
            aT, bvec, _, _ = ab_tiles[b]
            ht = h8pool.tile([128, 4, TT], f8, tag="ht", name=f"h8_{b}_{i}")
            for j in range(2):
                nc.scalar.activation(
                    ht[:, j, :], xbf[b][:, j, TT * i:TT * i + TT],
                    AF.Identity, bias=bvec[:, j:j + 1], scale=aT[:, j:j + 1])
            for j in range(2, 4):
                nc.vector.tensor_scalar(
                    ht[:, j, :], xbf[b][:, j, TT * i:TT * i + TT],
                    aT[:, j:j + 1], bvec[:, j:j + 1],
                    op0=ALU.mult, op1=ALU.add)
            return ht

        def emit_passA(b, pools, qk_shared, interleave_next, hook=None):
            """Emits passA for batch b. pools = (qps_pool, kps_pool, wps_pool).
            qk_shared: q/k PSUM tiles share one tag (tight PSUM phases).
            interleave_next: emit pass0 macros for batch b+1 inside the loop.
            hook(i): extra per-iteration emission (merged passB stream).
            Returns w_sb16 (block-diag packed bf16 w)."""
            qps_pool, kps_pool, wps_pool = pools
            qtag, ktag = ("qk", "qk") if qk_shared else ("q", "k")
            w_ps = wps_pool.tile([128, 4, 128], f32, name=f"wps{b}")
            ht_cur = emit_h8(b, 0)
            prev = None  # (eq, ek, i) pending w-accumulation
            for i in range(NMi):
                ht_next = emit_h8(b, i + 1) if i + 1 < NMi else None
                eq = wka.tile([128, 4, TT], bf, tag="wka", name=f"eq_{b}_{i}")
                ek = wka.tile([128, 4, TT], bf, tag="wka", name=f"ek_{b}_{i}")
                for s in range(4):
                    qps = qps_pool.tile([128, TT], f32, tag=qtag,
                                        name=f"qps_{b}_{i}_{s}")
                    kps = kps_pool.tile([128, TT], f32, tag=ktag,
                                        name=f"kps_{b}_{i}_{s}")
                    lhs0 = ht_cur[:, 0:2, 128 * s:128 * s + 128]
                    lhs1 = ht_cur[:, 2:4, 128 * s:128 * s + 128]
                    nc.tensor.matmul(qps[:], lhs0, wqt8[:, 0:2, :],
                                     perf_mode=DR, start=True, stop=False)
                    nc.tensor.matmul(qps[:], lhs1, wqt8[:, 2:4, :],
                                     perf_mode=DR, start=False, stop=True)
                    nc.tensor.matmul(kps[:], lhs0, wkt8[:, 0:2, :],
                                     perf_mode=DR, start=True, stop=False)
                    nc.tensor.matmul(kps[:], lhs1, wkt8[:, 2:4, :],
                                     perf_mode=DR, start=False, stop=True)
                    nc.scalar.activation(eq[:, s, :], qps[:], AF.Exp,
                                         scale=1.0 / WS)
                    nc.scalar.activation(ek[:, s, :], kps[:], AF.Exp,
                                         scale=1.0 / WS)
                # softmax denominators; kp folded into ek in place
                sq = smpool.tile([128, 4, NH], f32, tag="sq",
                                 name=f"sq_{b}_{i}")
                nc.vector.tensor_reduce(
                    sq[:], eq.rearrange("p s (n c) -> p (s n) c", c=HC),
                    axis=AX.X, op=ALU.add)
                sk = smpool.tile([128, 4, NH], f32, tag="sk",
                                 name=f"sk_{b}_{i}")
                nc.vector.tensor_reduce(
                    sk[:], ek.rearrange("p s (n c) -> p (s n) c", c=HC),
                    axis=AX.X, op=ALU.add)
                ss = smpool.tile([128, 4, NH], f32, tag="ss",
                                 name=f"ss_{b}_{i}")
                nc.vector.tensor_mul(ss[:], sq[:], sk[:])
                rr = smpool.tile([128, 4, NH], f32, tag="rr",
                                 name=f"rr_{b}_{i}")
                nc.vector.reciprocal(rr[:], ss[:])
                nc.gpsimd.tensor_mul(
                    ek.rearrange("p s (n c) -> p s n c", c=HC),
                    ek.rearrange("p s (n c) -> p s n c", c=HC),
                    rr[:, :, :, None].broadcast_to([128, 4, NH, HC]))
                # emit pending w-accumulation for i-1 (keeps PE fed: deps for
                # i-1 completed while qk(i) matmuls were running)
                if prev is not None:
                    emit_w(prev, w_ps, first=(prev[2] == 0), last=False)
                if interleave_next and b + 1 < B:
                    emit_pass0_macro(b + 1, i)
                prev = (eq, ek, i)
                if hook is not None:
                    hook(i)
                ht_cur = ht_next
            emit_w(prev, w_ps, first=False, last=True)
            # w finalize: PSUM -> SBUF bf16, then per-head 32x32 blocks into
            # block-diagonal w_sb16 (head h=4s+m at [32s, m, 32s])
            w_sc = wscpool.tile([128, 4, 128], bf, tag="wsc", name=f"wsc{b}")
            nc.vector.tensor_copy(w_sc[:], w_ps[:])
            w_sb = wsbpool.tile([128, 4, 128], bf, tag="wsb", name=f"wsb{b}")
            nc.vector.memset(w_sb[:], 0.0)
            for h in range(NH):
                s, m = h // 4, h % 4
                nc.sync.dma_start(
                    w_sb[32 * s:32 * s + 32, m, 32 * s:32 * s + 32],
                    w_sc[32 * m:32 * m + 32, s, 32 * m:32 * m + 32])
            return w_sb

        def emit_w(prev, w_ps, first, last):
            eq, ek, _ = prev
            for s in range(4):
                for m in range(4):
                    nc.tensor.matmul(
                        w_ps[:, m, :],
                        ek[:, s, 128 * m:128 * m + 128],
                        eq[:, s, 128 * m:128 * m + 128],
                        start=(first and s == 0 and m == 0),
                        stop=(last and s == 3 and m == 3),
                        skip_group_check=True)

        def emit_fold_bias(b, w_sb):
            """bp' = bp + Wp @ (w @ cv): fold the v bias all the way out."""
            _, _, _, cvP16 = ab_tiles[b]
            bpb = stpool.tile([128, 4], f32, tag="bpb", name=f"bpb{b}")
            with tc.tile_pool(name=f"fb_ps{b}", bufs=2, space="PSUM") as fps:
                h2b_ps = fps.tile([128, 4], f32, name=f"h2b{b}")
                for m in range(4):
                    nc.tensor.matmul(h2b_ps[:, m:m + 1], w_sb[:, m, :],
                                     cvP16[:, m:m + 1],
                                     start=(m == 0), stop=(m == 3),
                                     skip_group_check=True)
                h2b16 = stpool.tile([128, 4], bf, tag="h2b16",
                                    name=f"h2b16_{b}")
                nc.vector.tensor_copy(h2b16[:], h2b_ps[:])
                bp_ps = fps.tile([128, 4], f32, name=f"bpps{b}")
                for n in range(4):
                    for g in range(4):
                        nc.tensor.matmul(
                            bp_ps[:, n:n + 1],
                            wpt16[:, g, 128 * n:128 * n + 128],
                            h2b16[:, g:g + 1],
                            start=(n == 0 and g == 0),
                            stop=(n == 3 and g == 3),
                            skip_group_check=True)
                nc.vector.tensor_add(bpb[:], bp_ps[:], bpP[:])
            fold_tiles[b] = bpb

        def make_passB_pools(est_b, b, bufs=(2, 2, 2)):
            vps_pool = est_b.enter_context(
                tc.tile_pool(name=f"v_ps{b}", bufs=bufs[0], space="PSUM"))
            h2ps_pool = est_b.enter_context(
                tc.tile_pool(name=f"h2_ps{b}", bufs=bufs[1], space="PSUM"))
            pjps_pool = est_b.enter_context(
                tc.tile_pool(name=f"pj_ps{b}", bufs=bufs[2], space="PSUM"))
            return vps_pool, h2ps_pool, pjps_pool

        class PassB:
            """Software-pipelined passB emitter: per step() emits v(i),
            h2(i-1), proj(i-2) so the PE never waits on drains."""

            def __init__(self, b, w_sb, pools):
                self.b = b
                self.w_sb = w_sb
                self.vps_pool, self.h2ps_pool, self.pjps_pool = pools
                _, _, self.wv_f, _ = ab_tiles[b]
                self.bpb = fold_tiles[b]
                self.vsb = {}
                self.h2t = {}

            def step_v(self, i):
                b = self.b
                vsb = wkb.tile([128, 4, TT], bf, tag="wkb",
                               name=f"vsb_{b}_{i}")
                for m in range(4):
                    vps = self.vps_pool.tile([128, TT], f32, tag="v",
                                             name=f"vps_{b}_{i}_{m}")
                    for j in range(4):
                        nc.tensor.matmul(
                            vps[:], self.wv_f[:, j, 128 * m:128 * m + 128],
                            xbf[b][:, j, TT * i:TT * i + TT],
                            start=(j == 0), stop=(j == 3))
                    nc.scalar.activation(vsb[:, m, :], vps[:], AF.Copy)
                self.vsb[i] = vsb

            def step_h2(self, i):
                b = self.b
                vsb = self.vsb.pop(i)
                h2t = wkb.tile([128, 4, TT], bf, tag="wkb",
                               name=f"h2_{b}_{i}")
                for m in range(4):
                    h2ps = self.h2ps_pool.tile([128, TT], f32, tag="h2p",
                                               name=f"h2ps_{b}_{i}_{m}")
                    nc.tensor.matmul(h2ps[:], self.w_sb[:, m, :], vsb[:, m, :])
                    if m % 2 == 0:
                        nc.scalar.copy(h2t[:, m, :], h2ps[:])
                    else:
                        nc.vector.tensor_copy(h2t[:, m, :], h2ps[:])
                self.h2t[i] = h2t

            def step_pj(self, i):
                b = self.b
                h2t = self.h2t.pop(i)
                ot = xin.tile([128, 4, TT], f32, tag="xt", name=f"ot_{b}_{i}")
                for n in range(4):
                    pj = self.pjps_pool.tile([128, TT], f32, tag="pj",
                                             name=f"pj_{b}_{i}_{n}")
                    for g in range(4):
                        nc.tensor.matmul(
                            pj[:], wpt16[:, g, 128 * n:128 * n + 128],
                            h2t[:, g, :],
                            start=(g == 0), stop=(g == 3))
                    nc.vector.scalar_tensor_tensor(
                        ot[:, n, :], in0=pj[:], scalar=self.bpb[:, n:n + 1],
                        in1=xbf[b][:, n, TT * i:TT * i + TT],
                        op0=ALU.add, op1=ALU.add)
                nc.sync.dma_start(
                    out_d[b, :, TT * i:TT * i + TT].rearrange(
                        "(j p) t -> p j t", p=128),
                    ot[:])

            def steps_for(self, i):
                # emit in dependency-distance order
                if i < NMi:
                    self.step_v(i)
                if 0 <= i - 1 < NMi:
                    self.step_h2(i - 1)
                if 0 <= i - 2 < NMi:
                    self.step_pj(i - 2)

        # ================= schedule =================
        for i in range(NMi):
            emit_pass0_macro(0, i)
        emit_finalize(0)

        with ExitStack() as est_a:
            qps_pool = est_a.enter_context(
                tc.tile_pool(name="q_ps0", bufs=3, space="PSUM"))
            kps_pool = est_a.enter_context(
                tc.tile_pool(name="k_ps0", bufs=3, space="PSUM"))
            wps_pool = est_a.enter_context(
                tc.tile_pool(name="w_ps0", bufs=1, space="PSUM"))
            w_sb0 = emit_passA(0, (qps_pool, kps_pool, wps_pool),
                               qk_shared=False, interleave_next=True)
        emit_finalize(1)

        with ExitStack() as est_m:
            # merged phase: passB(0) || passA(1).
            # PSUM: fold_bias transient, then qk(3) + w(1) + v(2) + h2(1)
            # + pj(1) = 8 banks.
            emit_fold_bias(0, w_sb0)
            qkps_pool = est_m.enter_context(
                tc.tile_pool(name="qk_ps1", bufs=3, space="PSUM"))
            wps_pool = est_m.enter_context(
                tc.tile_pool(name="w_ps1", bufs=1, space="PSUM"))
            pb_pools = make_passB_pools(est_m, 0, bufs=(2, 1, 1))
            pb0 = PassB(0, w_sb0, pb_pools)
            w_sb1 = emit_passA(1, (qkps_pool, qkps_pool, wps_pool),
                               qk_shared=True, interleave_next=False,
                               hook=pb0.steps_for)
            pb0.steps_for(NMi)      # h2(NMi-1), pj(NMi-2)
            pb0.steps_for(NMi + 1)  # pj(NMi-1)

        with ExitStack() as est_b:
            emit_fold_bias(1, w_sb1)
            pb_pools = make_passB_pools(est_b, 1, bufs=(2, 2, 2))
            pb1 = PassB(1, w_sb1, pb_pools)
            for i in range(NMi + 2):
                pb1.steps_for(i)

    nc.compile()
    return nc


def _host_prep(x, gn_scale, gn_bias, wq, bq, wk, bk, wv, bv, wp, bp):
    assert not np.any(bq) and not np.any(bk), "q/k biases must be zero"
    pv = _head_perm_v()
    sel = np.zeros((128, 8), dtype=np.float32)
    for p in range(128):
        sel[p, p // CG] = 1.0
    consts = {
        "wqt8": np.ascontiguousarray(wq.T * WS).astype(F8NP),
        "wkt8": np.ascontiguousarray(wk.T * WS).astype(F8NP),
        "wvt16": np.ascontiguousarray(wv.T[:, pv]).astype(BF16NP),
        "wpt16": np.ascontiguousarray(wp.T[pv, :]).astype(BF16NP),
        "gammaP": _to_part4(gn_scale).astype(np.float32),
        "betaP": _to_part4(gn_bias).astype(np.float32),
        "bvP": _to_part4(bv[pv]).astype(np.float32),
        "bpP": _to_part4(bp).astype(np.float32),
        "sel": sel,
        "selT": np.ascontiguousarray(sel.T),
    }
    return consts


_NC_CACHE = {}


def kernel(x, gn_scale, gn_bias, wq, bq, wk, bk, wv, bv, wp, bp):
    from concourse.bass_utils import run_bass_kernel_spmd

    x = np.asarray(x, dtype=np.float32)
    consts = _host_prep(x, np.asarray(gn_scale), np.asarray(gn_bias),
                        np.asarray(wq), np.asarray(bq), np.asarray(wk),
                        np.asarray(bk), np.asarray(wv), np.asarray(bv),
                        np.asarray(wp), np.asarray(bp))

    key = (B_SHARD, T_FULL)
    if key not in _NC_CACHE:
        _NC_CACHE[key] = build_nc(B_SHARD, T_FULL)
    nc = _NC_CACHE[key]

    in_maps = []
    for c in range(N_CORES):
        m = dict(consts)
        m["x"] = np.ascontiguousarray(x[B_SHARD * c:B_SHARD * (c + 1)])
        in_maps.append(m)
    res = run_bass_kernel_spmd(nc, in_maps, core_ids=list(range(N_CORES)))
    out = np.concatenate([r["out"] for r in res.results], axis=0)
    return out.astype(np.float32)


# revision 15
# speedup vs baseline: 1.2133x; 1.1146x over previous
"""Trainium2 Bass kernel for nn_ChannelAttnBlock (GroupNorm + channel attention).

Self-contained: takes FULL unsharded inputs, shards batch over 8 NeuronCores
(2 batches/core), runs one SPMD NEFF, gathers the full output.

Per-core dataflow (B=2 batches, C=512, T=8192), v2:
  pass 0: stream x (fp32) once from HBM; bn_stats for GroupNorm; cast x to a
          bf16 SBUF-resident copy (xbf). No further HBM reads of x.
  finalize: tiny selector matmuls aggregate 32 GN groups -> per-channel
          affine a, b; fold a into bf16 v-weights (wv_f) and compute the
          folded v bias cv = Wv^T b + bv.
  pass A: h8 = fp8(a*xbf+b) on gpsimd; q/k GEMMs as fp8 DoubleRow matmuls
          (K=256 per instr); exp on ACT with output in bf16 (weight
          prescale WS undone via the ACT scale); channel softmax sums are
          free-dim segment reduces; kp = ek/(Sq*Sk) in place; w accumulated
          in PSUM with bf16 N=128 matmuls.
  pass B: v = wv_f^T xbf (bf16 matmuls); h2 = w @ v via block-diag packed
          bf16 w; out = Wp h2 + bp' + xbf with all biases pre-folded into
          bp' = bp + Wp(w@cv).
  schedule: pass0(0); passA(0) || pass0(1); passB(0) || passA(1); passB(1).
  PE emission is software-pipelined (w-mm of i-1 after qk of i, etc.) so the
  tensor engine never waits on ACT/DVE.
"""

import numpy as np
import ml_dtypes

C = 512
NH = 16      # heads
HC = 32      # channels/head
G = 32       # groupnorm groups
CG = C // G  # 16 channels per group
EPS = 1e-6
WS = 32.0    # q/k weight prescale before fp8 cast (undone in exp ACT)

N_CORES = 8
B_FULL = 16
T_FULL = 8192
B_SHARD = B_FULL // N_CORES  # 2
TT = 512                     # t macro-tile
NM = T_FULL // TT            # 16 macros per batch

F8NP = ml_dtypes.float8_e4m3
BF16NP = ml_dtypes.bfloat16


def _head_perm_v():
    # v-ctile m holds heads (m, m+4, m+8, m+12) at 32-row slots 0..3
    pv = np.zeros(C, dtype=np.int64)
    for h in range(NH):
        m, s = h % 4, h // 4
        pv[128 * m + 32 * s: 128 * m + 32 * s + 32] = np.arange(32 * h, 32 * h + 32)
    return pv


def _to_part4(vec):
    # [512] -> [128, 4]: column j = channels 128j..128j+127
    return np.ascontiguousarray(vec.reshape(4, 128).T)


def build_nc(B, T, debug=False):
    import concourse.tile as tile
    import concourse.mybir as mybir
    from concourse import bacc

    NMi = T // TT
    f32 = mybir.dt.float32
    f32r = mybir.dt.float32r
    f8 = mybir.dt.float8e4
    bf = mybir.dt.bfloat16
    AF = mybir.ActivationFunctionType
    ALU = mybir.AluOpType
    AX = mybir.AxisListType
    DR = mybir.MatmulPerfMode.DoubleRow

    nc = bacc.Bacc("TRN2", target_bir_lowering=False, debug=debug)

    x_d = nc.dram_tensor("x16", [B, C, T], bf, kind="ExternalInput").ap()
    wqt8_d = nc.dram_tensor("wqt8", [C, C], f8, kind="ExternalInput").ap()
    wkt8_d = nc.dram_tensor("wkt8", [C, C], f8, kind="ExternalInput").ap()
    wvt16_d = nc.dram_tensor("wvt16", [C, C], bf, kind="ExternalInput").ap()
    wpt16_d = nc.dram_tensor("wpt16", [C, C], bf, kind="ExternalInput").ap()
    gammaP_d = nc.dram_tensor("gammaP", [128, 4], f32, kind="ExternalInput").ap()
    betaP_d = nc.dram_tensor("betaP", [128, 4], f32, kind="ExternalInput").ap()
    bvP_d = nc.dram_tensor("bvP", [128, 4], f32, kind="ExternalInput").ap()
    bpP_d = nc.dram_tensor("bpP", [128, 4], f32, kind="ExternalInput").ap()
    sel_d = nc.dram_tensor("sel", [128, 8], f32, kind="ExternalInput").ap()
    selT_d = nc.dram_tensor("selT", [8, 128], f32, kind="ExternalInput").ap()
    out_d = nc.dram_tensor("out", [B, C, T], f32, kind="ExternalOutput").ap()

    def r(ap):
        return ap.bitcast(f32r)

    from contextlib import ExitStack

    with tile.TileContext(nc) as tc, ExitStack() as est:
        p = lambda name, bufs: est.enter_context(
            tc.tile_pool(name=name, bufs=bufs))
        wpool = p("wpool", 1)
        cpool = p("cpool", 1)
        xbfpool = p("xbfpool", 1)
        stpool = p("stpool", 2)
        xin = p("xin", 2)       # fp32 staging: pass0 in, passB out
        h8pool = p("h8pool", 2)
        wka = p("wka", 5)       # passA bf16 work tiles (eq, ek)
        wkb = p("wkb", 3)       # passB bf16 work tiles (vsb, h2b)
        smpool = p("smpool", 2)
        wsbpool = p("wsbpool", 2)
        wscpool = p("wscpool", 1)
        wfpool = p("wfpool", 1)

        # ---- load weights & constants ----
        wqt8 = wpool.tile([128, 4, C], f8)
        wkt8 = wpool.tile([128, 4, C], f8)
        wvt16 = wpool.tile([128, 4, C], bf)
        wpt16 = wpool.tile([128, 4, C], bf)
        for j in range(4):
            nc.sync.dma_start(wqt8[:, j, :], wqt8_d[128 * j:128 * j + 128, :])
            nc.sync.dma_start(wkt8[:, j, :], wkt8_d[128 * j:128 * j + 128, :])
            nc.sync.dma_start(wvt16[:, j, :], wvt16_d[128 * j:128 * j + 128, :])
            nc.sync.dma_start(wpt16[:, j, :], wpt16_d[128 * j:128 * j + 128, :])
        gammaP = cpool.tile([128, 4], f32)
        betaP = cpool.tile([128, 4], f32)
        bvP = cpool.tile([128, 4], f32)
        bpP = cpool.tile([128, 4], f32)
        sel_sb = cpool.tile([128, 8], f32)
        selT_sb = cpool.tile([8, 128], f32)
        nc.sync.dma_start(gammaP[:], gammaP_d)
        nc.sync.dma_start(betaP[:], betaP_d)
        nc.sync.dma_start(bvP[:], bvP_d)
        nc.sync.dma_start(bpP[:], bpP_d)
        nc.sync.dma_start(sel_sb[:], sel_d)
        nc.sync.dma_start(selT_sb[:], selT_d)
        eps_t = cpool.tile([8, 1], f32)
        nc.vector.memset(eps_t[:], EPS)

        xbf = [xbfpool.tile([128, 4, T], bf, name=f"xbf{b}") for b in range(B)]

        bn_tiles = {}
        ab_tiles = {}
        fold_tiles = {}

        def x_macro_ap(b, i):
            return x_d[b, :, TT * i:TT * i + TT].rearrange(
                "(j p) t -> p j t", p=128)

        def emit_pass0_macro(b, i):
            if b not in bn_tiles:
                bnall = stpool.tile([128, 4, NMi * 6], f32, tag="bnall",
                                    name=f"bnall{b}")
                bn_tiles[b] = bnall
            bnall = bn_tiles[b]
            nc.sync.dma_start(xbf[b][:, :, TT * i:TT * i + TT],
                              x_macro_ap(b, i))
            for j in range(4):
                nc.vector.bn_stats(bnall[:, j, 6 * i:6 * i + 6],
                                   xbf[b][:, j, TT * i:TT * i + TT])

        def emit_finalize(b):
            bnall = bn_tiles[b]
            statsc = stpool.tile([128, 4, 2], f32, tag="statsc",
                                 name=f"statsc{b}")
            stats2 = stpool.tile([128, 8], f32, tag="stats2",
                                 name=f"stats2_{b}")
            for j in range(4):
                nc.vector.bn_aggr(statsc[:, j, :], bnall[:, j, :])
                nc.vector.tensor_copy(stats2[:, 2 * j:2 * j + 1],
                                      statsc[:, j, 0:1])
                nc.vector.scalar_tensor_tensor(
                    stats2[:, 2 * j + 1:2 * j + 2],
                    in0=statsc[:, j, 0:1], scalar=statsc[:, j, 0:1],
                    in1=statsc[:, j, 1:2], op0=ALU.mult, op1=ALU.add)
            aT = stpool.tile([128, 4], f32, tag="aT", name=f"aT{b}")
            bvec = stpool.tile([128, 4], f32, tag="bvec", name=f"bvec{b}")
            with tc.tile_pool(name=f"st_ps{b}", bufs=2, space="PSUM") as stps:
                gsum_ps = stps.tile([8, 8], f32, name=f"gsum{b}")
                nc.tensor.matmul(gsum_ps[:], sel_sb[:], stats2[:])
                gs = stpool.tile([8, 4, 2], f32, tag="gs", name=f"gs{b}")
                nc.vector.tensor_scalar_mul(gs[:], gsum_ps.rearrange(
                    "p (j s) -> p j s", s=2), 1.0 / CG)
                mg2 = stpool.tile([8, 4], f32, tag="mg2", name=f"mg2_{b}")
                nc.vector.tensor_mul(mg2[:], gs[:, :, 0], gs[:, :, 0])
                gvar = stpool.tile([8, 4], f32, tag="gvar", name=f"gvar{b}")
                nc.vector.tensor_sub(gvar[:], gs[:, :, 1], mg2[:])
                gstd = stpool.tile([8, 4], f32, tag="gstd", name=f"gstd{b}")
                nc.scalar.activation(gstd[:], gvar[:], AF.Sqrt, bias=eps_t[:])
                ginv = stpool.tile([8, 4], f32, tag="ginv", name=f"ginv{b}")
                nc.vector.reciprocal(ginv[:], gstd[:])
                gb = stpool.tile([8, 4, 2], f32, tag="gb", name=f"gb{b}")
                nc.vector.tensor_copy(gb[:, :, 0], gs[:, :, 0])
                nc.vector.tensor_copy(gb[:, :, 1], ginv[:])
                chB_ps = stps.tile([128, 8], f32, name=f"chB{b}")
                nc.tensor.matmul(chB_ps[:], selT_sb[:], gb.rearrange(
                    "p j s -> p (j s)"))
                chB = chB_ps.rearrange("p (j s) -> p j s", s=2)
                nc.vector.tensor_mul(aT[:], gammaP[:], chB[:, :, 1])
                tmpb = stpool.tile([128, 4], f32, tag="tmpb", name=f"tmpb{b}")
                nc.vector.tensor_mul(tmpb[:], chB[:, :, 0], aT[:])
                nc.vector.tensor_sub(bvec[:], betaP[:], tmpb[:])
            # fold GN affine into v-weights: wv_f = wvt16 * a (per in-ch row)
            wv_f = wfpool.tile([128, 4, C], bf, tag="wvf", name=f"wvf{b}")
            for j in range(4):
                nc.vector.tensor_scalar_mul(wv_f[:, j, :], wvt16[:, j, :],
                                            aT[:, j:j + 1])
            # cv = Wv^T b + bv (per v-channel), bf16 copy for tiny matmuls
            bvec16 = stpool.tile([128, 4], bf, tag="bvec16", name=f"bv16_{b}")
            nc.vector.tensor_copy(bvec16[:], bvec[:])
            cvP16 = stpool.tile([128, 4], bf, tag="cvP16", name=f"cvP16_{b}")
            with tc.tile_pool(name=f"cv_ps{b}", bufs=1, space="PSUM") as cvps:
                cv_ps = cvps.tile([128, 4], f32, name=f"cvp{b}")
                for m in range(4):
                    for j in range(4):
                        nc.tensor.matmul(
                            cv_ps[:, m:m + 1],
                            wvt16[:, j, 128 * m:128 * m + 128],
                            bvec16[:, j:j + 1],
                            start=(j == 0 and m == 0),
                            stop=(j == 3 and m == 3),
                            skip_group_check=True)
                cv_f = stpool.tile([128, 4], f32, tag="cvf", name=f"cvf{b}")
                nc.vector.tensor_add(cv_f[:], cv_ps[:], bvP[:])
                nc.vector.tensor_copy(cvP16[:], cv_f[:])
            ab_tiles[b] = (aT, bvec, wv_f, cvP16)

        def emit_h8(b, i):
            aT, bvec, _, _ = ab_tiles[b]
            ht = h8pool.tile([128, 4, TT], f8, tag="ht", name=f"h8_{b}_{i}")
            for j in range(2):
                nc.scalar.activation(
                    ht[:, j, :], xbf[b][:, j, TT * i:TT * i + TT],
                    AF.Identity, bias=bvec[:, j:j + 1], scale=aT[:, j:j + 1])
            for j in range(2, 4):
                nc.gpsimd.tensor_scalar(
                    ht[:, j, :], xbf[b][:, j, TT * i:TT * i + TT],
                    aT[:, j:j + 1], bvec[:, j:j + 1],
                    op0=ALU.mult, op1=ALU.add)
            return ht

        def emit_passA(b, pools, qk_shared, interleave_next, hook=None,
                       exp2=False):
            """Emits passA for batch b. pools = (qps_pool, kps_pool, wps_pool).
            qk_shared: q/k PSUM tiles share one tag (tight PSUM phases).
            interleave_next: emit pass0 macros for batch b+1 inside the loop.
            hook(i): extra per-iteration emission (merged passB stream).
            exp2: 2-bank PSUM tiles + [128,1024] exp ACTs (needs PSUM room).
            Returns w_sb16 (block-diag packed bf16 w)."""
            qps_pool, kps_pool, wps_pool = pools
            qtag, ktag = ("qk", "qk") if qk_shared else ("q", "k")
            w_ps = wps_pool.tile([128, 4, 128], f32, name=f"wps{b}")
            ht_cur = emit_h8(b, 0)
            prev = None  # (eq, ek, i) pending w-accumulation
            for i in range(NMi):
                ht_next = emit_h8(b, i + 1) if i + 1 < NMi else None
                eq = wka.tile([128, 4, TT], bf, tag="wka", name=f"eq_{b}_{i}")
                ek = wka.tile([128, 4, TT], bf, tag="wka", name=f"ek_{b}_{i}")
                if exp2:
                    for sp in range(2):
                        qps = qps_pool.tile([128, 2, TT], f32, tag=qtag,
                                            name=f"qps_{b}_{i}_{sp}")
                        kps = kps_pool.tile([128, 2, TT], f32, tag=ktag,
                                            name=f"kps_{b}_{i}_{sp}")
                        for s2 in range(2):
                            s = 2 * sp + s2
                            lhs0 = ht_cur[:, 0:2, 128 * s:128 * s + 128]
                            lhs1 = ht_cur[:, 2:4, 128 * s:128 * s + 128]
                            nc.tensor.matmul(qps[:, s2, :], lhs0,
                                             wqt8[:, 0:2, :], perf_mode=DR,
                                             start=True, stop=False)
                            nc.tensor.matmul(qps[:, s2, :], lhs1,
                                             wqt8[:, 2:4, :], perf_mode=DR,
                                             start=False, stop=True)
                            nc.tensor.matmul(kps[:, s2, :], lhs0,
                                             wkt8[:, 0:2, :], perf_mode=DR,
                                             start=True, stop=False)
                            nc.tensor.matmul(kps[:, s2, :], lhs1,
                                             wkt8[:, 2:4, :], perf_mode=DR,
                                             start=False, stop=True)
                        nc.scalar.activation(eq[:, 2 * sp:2 * sp + 2, :],
                                             qps[:], AF.Exp, scale=1.0 / WS)
                        nc.scalar.activation(ek[:, 2 * sp:2 * sp + 2, :],
                                             kps[:], AF.Exp, scale=1.0 / WS)
                else:
                    for s in range(4):
                        qps = qps_pool.tile([128, TT], f32, tag=qtag,
                                            name=f"qps_{b}_{i}_{s}")
                        kps = kps_pool.tile([128, TT], f32, tag=ktag,
                                            name=f"kps_{b}_{i}_{s}")
                        lhs0 = ht_cur[:, 0:2, 128 * s:128 * s + 128]
                        lhs1 = ht_cur[:, 2:4, 128 * s:128 * s + 128]
                        nc.tensor.matmul(qps[:], lhs0, wqt8[:, 0:2, :],
                                         perf_mode=DR, start=True, stop=False)
                        nc.tensor.matmul(qps[:], lhs1, wqt8[:, 2:4, :],
                                         perf_mode=DR, start=False, stop=True)
                        nc.tensor.matmul(kps[:], lhs0, wkt8[:, 0:2, :],
                                         perf_mode=DR, start=True, stop=False)
                        nc.tensor.matmul(kps[:], lhs1, wkt8[:, 2:4, :],
                                         perf_mode=DR, start=False, stop=True)
                        nc.scalar.activation(eq[:, s, :], qps[:], AF.Exp,
                                             scale=1.0 / WS)
                        nc.scalar.activation(ek[:, s, :], kps[:], AF.Exp,
                                             scale=1.0 / WS)
                # softmax denominators; kp folded into ek in place
                sq = smpool.tile([128, 4, NH], f32, tag="sq",
                                 name=f"sq_{b}_{i}")
                nc.vector.tensor_reduce(
                    sq[:], eq.rearrange("p s (n c) -> p (s n) c", c=HC),
                    axis=AX.X, op=ALU.add)
                sk = smpool.tile([128, 4, NH], f32, tag="sk",
                                 name=f"sk_{b}_{i}")
                nc.vector.tensor_reduce(
                    sk[:], ek.rearrange("p s (n c) -> p (s n) c", c=HC),
                    axis=AX.X, op=ALU.add)
                ss = smpool.tile([128, 4, NH], f32, tag="ss",
                                 name=f"ss_{b}_{i}")
                nc.vector.tensor_mul(ss[:], sq[:], sk[:])
                rr = smpool.tile([128, 4, NH], f32, tag="rr",
                                 name=f"rr_{b}_{i}")
                nc.vector.reciprocal(rr[:], ss[:])
                nc.gpsimd.tensor_mul(
                    ek.rearrange("p s (n c) -> p s n c", c=HC),
                    ek.rearrange("p s (n c) -> p s n c", c=HC),
                    rr[:, :, :, None].broadcast_to([128, 4, NH, HC]))
                # emit pending w-accumulation for i-1 (keeps PE fed: deps for
                # i-1 completed while qk(i) matmuls were running)
                if prev is not None:
                    emit_w(prev, w_ps, first=(prev[2] == 0), last=False)
                if interleave_next and b + 1 < B:
                    emit_pass0_macro(b + 1, i)
                prev = (eq, ek, i)
                if hook is not None:
                    hook(i)
                ht_cur = ht_next
            emit_w(prev, w_ps, first=False, last=True)
            # w finalize: PSUM -> SBUF bf16, then per-head 32x32 blocks into
            # block-diagonal w_sb16 (head h=4s+m at [32s, m, 32s])
            w_sc = wscpool.tile([128, 4, 128], bf, tag="wsc", name=f"wsc{b}")
            nc.vector.tensor_copy(w_sc[:], w_ps[:])
            w_sb = wsbpool.tile([128, 4, 128], bf, tag="wsb", name=f"wsb{b}")
            nc.vector.memset(w_sb[:], 0.0)
            for h in range(NH):
                s, m = h // 4, h % 4
                nc.sync.dma_start(
                    w_sb[32 * s:32 * s + 32, m, 32 * s:32 * s + 32],
                    w_sc[32 * m:32 * m + 32, s, 32 * m:32 * m + 32])
            return w_sb

        def emit_w(prev, w_ps, first, last):
            eq, ek, _ = prev
            for s in range(4):
                for m in range(4):
                    nc.tensor.matmul(
                        w_ps[:, m, :],
                        ek[:, s, 128 * m:128 * m + 128],
                        eq[:, s, 128 * m:128 * m + 128],
                        start=(first and s == 0 and m == 0),
                        stop=(last and s == 3 and m == 3),
                        skip_group_check=True)

        def emit_fold_bias(b, w_sb):
            """bp' = bp + Wp @ (w @ cv): fold the v bias all the way out."""
            _, _, _, cvP16 = ab_tiles[b]
            bpb = stpool.tile([128, 4], f32, tag="bpb", name=f"bpb{b}")
            with tc.tile_pool(name=f"fb_ps{b}", bufs=2, space="PSUM") as fps:
                h2b_ps = fps.tile([128, 4], f32, name=f"h2b{b}")
                for m in range(4):
                    nc.tensor.matmul(h2b_ps[:, m:m + 1], w_sb[:, m, :],
                                     cvP16[:, m:m + 1],
                                     start=(m == 0), stop=(m == 3),
                                     skip_group_check=True)
                h2b16 = stpool.tile([128, 4], bf, tag="h2b16",
                                    name=f"h2b16_{b}")
                nc.vector.tensor_copy(h2b16[:], h2b_ps[:])
                bp_ps = fps.tile([128, 4], f32, name=f"bpps{b}")
                for n in range(4):
                    for g in range(4):
                        nc.tensor.matmul(
                            bp_ps[:, n:n + 1],
                            wpt16[:, g, 128 * n:128 * n + 128],
                            h2b16[:, g:g + 1],
                            start=(n == 0 and g == 0),
                            stop=(n == 3 and g == 3),
                            skip_group_check=True)
                nc.vector.tensor_add(bpb[:], bp_ps[:], bpP[:])
            fold_tiles[b] = bpb

        def make_passB_pools(est_b, b, bufs=(2, 2, 2)):
            vps_pool = est_b.enter_context(
                tc.tile_pool(name=f"v_ps{b}", bufs=bufs[0], space="PSUM"))
            h2ps_pool = est_b.enter_context(
                tc.tile_pool(name=f"h2_ps{b}", bufs=bufs[1], space="PSUM"))
            pjps_pool = est_b.enter_context(
                tc.tile_pool(name=f"pj_ps{b}", bufs=bufs[2], space="PSUM"))
            return vps_pool, h2ps_pool, pjps_pool

        class PassB:
            """Software-pipelined passB emitter: per step() emits v(i),
            h2(i-1), proj(i-2) so the PE never waits on drains."""

            def __init__(self, b, w_sb, pools):
                self.b = b
                self.w_sb = w_sb
                self.vps_pool, self.h2ps_pool, self.pjps_pool = pools
                _, _, self.wv_f, _ = ab_tiles[b]
                self.bpb = fold_tiles[b]
                self.vsb = {}
                self.h2t = {}

            def step_v(self, i):
                b = self.b
                vsb = wkb.tile([128, 4, TT], bf, tag="wkb",
                               name=f"vsb_{b}_{i}")
                for m in range(4):
                    vps = self.vps_pool.tile([128, TT], f32, tag="v",
                                             name=f"vps_{b}_{i}_{m}")
                    for j in range(4):
                        nc.tensor.matmul(
                            vps[:], self.wv_f[:, j, 128 * m:128 * m + 128],
                            xbf[b][:, j, TT * i:TT * i + TT],
                            start=(j == 0), stop=(j == 3))
                    nc.scalar.activation(vsb[:, m, :], vps[:], AF.Copy)
                self.vsb[i] = vsb

            def step_h2(self, i):
                b = self.b
                vsb = self.vsb.pop(i)
                h2t = wkb.tile([128, 4, TT], bf, tag="wkb",
                               name=f"h2_{b}_{i}")
                for m in range(4):
                    h2ps = self.h2ps_pool.tile([128, TT], f32, tag="h2p",
                                               name=f"h2ps_{b}_{i}_{m}")
                    nc.tensor.matmul(h2ps[:], self.w_sb[:, m, :], vsb[:, m, :])
                    if m % 2 == 0:
                        nc.scalar.copy(h2t[:, m, :], h2ps[:])
                    else:
                        nc.vector.tensor_copy(h2t[:, m, :], h2ps[:])
                self.h2t[i] = h2t

            def step_pj(self, i):
                b = self.b
                h2t = self.h2t.pop(i)
                ot = xin.tile([128, 4, TT], f32, tag="xt", name=f"ot_{b}_{i}")
                for n in range(4):
                    pj = self.pjps_pool.tile([128, TT], f32, tag="pj",
                                             name=f"pj_{b}_{i}_{n}")
                    for g in range(4):
                        nc.tensor.matmul(
                            pj[:], wpt16[:, g, 128 * n:128 * n + 128],
                            h2t[:, g, :],
                            start=(g == 0), stop=(g == 3))
                    nc.vector.scalar_tensor_tensor(
                        ot[:, n, :], in0=pj[:], scalar=self.bpb[:, n:n + 1],
                        in1=xbf[b][:, n, TT * i:TT * i + TT],
                        op0=ALU.add, op1=ALU.add)
                nc.sync.dma_start(
                    out_d[b, :, TT * i:TT * i + TT].rearrange(
                        "(j p) t -> p j t", p=128),
                    ot[:])

            def steps_for(self, i):
                # emit in dependency-distance order
                if i < NMi:
                    self.step_v(i)
                if 0 <= i - 1 < NMi:
                    self.step_h2(i - 1)
                if 0 <= i - 2 < NMi:
                    self.step_pj(i - 2)

        # ================= schedule =================
        for i in range(NMi):
            emit_pass0_macro(0, i)
        emit_finalize(0)

        with ExitStack() as est_a:
            qkps_pool0 = est_a.enter_context(
                tc.tile_pool(name="qk_ps0", bufs=3, space="PSUM"))
            wps_pool = est_a.enter_context(
                tc.tile_pool(name="w_ps0", bufs=1, space="PSUM"))
            w_sb0 = emit_passA(0, (qkps_pool0, qkps_pool0, wps_pool),
                               qk_shared=True, interleave_next=True,
                               exp2=True)
        emit_finalize(1)

        with ExitStack() as est_m:
            # merged phase: passB(0) || passA(1).
            # PSUM: fold_bias transient, then qk(3) + w(1) + v(2) + h2(1)
            # + pj(1) = 8 banks.
            emit_fold_bias(0, w_sb0)
            qkps_pool = est_m.enter_context(
                tc.tile_pool(name="qk_ps1", bufs=3, space="PSUM"))
            wps_pool = est_m.enter_context(
                tc.tile_pool(name="w_ps1", bufs=1, space="PSUM"))
            pb_pools = make_passB_pools(est_m, 0, bufs=(2, 1, 1))
            pb0 = PassB(0, w_sb0, pb_pools)
            w_sb1 = emit_passA(1, (qkps_pool, qkps_pool, wps_pool),
                               qk_shared=True, interleave_next=False,
                               hook=pb0.steps_for)
            pb0.steps_for(NMi)      # h2(NMi-1), pj(NMi-2)
            pb0.steps_for(NMi + 1)  # pj(NMi-1)

        with ExitStack() as est_b:
            emit_fold_bias(1, w_sb1)
            pb_pools = make_passB_pools(est_b, 1, bufs=(2, 2, 2))
            pb1 = PassB(1, w_sb1, pb_pools)
            for i in range(NMi + 2):
                pb1.steps_for(i)

    nc.compile()
    return nc


def _host_prep(x, gn_scale, gn_bias, wq, bq, wk, bk, wv, bv, wp, bp):
    assert not np.any(bq) and not np.any(bk), "q/k biases must be zero"
    pv = _head_perm_v()
    sel = np.zeros((128, 8), dtype=np.float32)
    for p in range(128):
        sel[p, p // CG] = 1.0
    consts = {
        "wqt8": np.ascontiguousarray(wq.T * WS).astype(F8NP),
        "wkt8": np.ascontiguousarray(wk.T * WS).astype(F8NP),
        "wvt16": np.ascontiguousarray(wv.T[:, pv]).astype(BF16NP),
        "wpt16": np.ascontiguousarray(wp.T[pv, :]).astype(BF16NP),
        "gammaP": _to_part4(gn_scale).astype(np.float32),
        "betaP": _to_part4(gn_bias).astype(np.float32),
        "bvP": _to_part4(bv[pv]).astype(np.float32),
        "bpP": _to_part4(bp).astype(np.float32),
        "sel": sel,
        "selT": np.ascontiguousarray(sel.T),
    }
    return consts


_NC_CACHE = {}


def kernel(x, gn_scale, gn_bias, wq, bq, wk, bk, wv, bv, wp, bp):
    from concourse.bass_utils import run_bass_kernel_spmd

    x = np.asarray(x, dtype=np.float32)
    consts = _host_prep(x, np.asarray(gn_scale), np.asarray(gn_bias),
                        np.asarray(wq), np.asarray(bq), np.asarray(wk),
                        np.asarray(bk), np.asarray(wv), np.asarray(bv),
                        np.asarray(wp), np.asarray(bp))
    x16 = x.astype(BF16NP)

    key = (B_SHARD, T_FULL)
    if key not in _NC_CACHE:
        _NC_CACHE[key] = build_nc(B_SHARD, T_FULL)
    nc = _NC_CACHE[key]

    in_maps = []
    for c in range(N_CORES):
        m = dict(consts)
        m["x16"] = np.ascontiguousarray(x16[B_SHARD * c:B_SHARD * (c + 1)])
        in_maps.append(m)
    res = run_bass_kernel_spmd(nc, in_maps, core_ids=list(range(N_CORES)))
    out = np.concatenate([r["out"] for r in res.results], axis=0)
    return out.astype(np.float32)
